# revision 21
# baseline (speedup 1.0000x reference)
"""Causal multi-head attention block (qkv proj + causal softmax attention + out proj)
for Trainium2, sharded over 8 NeuronCores: data-parallel over batch (2) x
tensor-parallel over heads (4 heads per core of 16).

Each core computes, for its batch b and its 4 heads:
  qT,kT [hd, S] and v [S, hd]  (qkv projection, weights pre-transposed on host)
  ST    [k, q] = kT.T-scores transposed, causal-blocked
  P     = exp(ST + mask)
  attnT [hd+1, q] = [v | 1].T @ P   (row hd = softmax denominator)
  attn_n = attnT / denom
  out_partial [S, D] = attn_n.T @ owT  (row-parallel out proj)
Host sums the 4 per-core partials of each batch.
"""

import os
import sys

import numpy as np

sys.path.insert(0, "/opt/trn_rl_repo")

import concourse.bass as bass
import concourse.tile as tile
from concourse import bacc, mybir
from concourse.bass import MemorySpace
from concourse.bass_utils import run_bass_kernel_spmd

F32 = mybir.dt.float32
F32R = mybir.dt.float32r
EXP = mybir.ActivationFunctionType.Exp
LN = mybir.ActivationFunctionType.Ln

B, S, D = 2, 2048, 1024
H, HD = 16, 64
NCORES = 8
NH = 4          # heads per core
NP = 2          # head pairs per core
SCALE = HD ** -0.5

N_DT = D // 128          # 8 d-tiles of 128
N_ST = S // 128          # 16 seq tiles of 128
N_CH = S // 512          # 4 seq chunks of 512
FQK = 2 * NH * HD // 128  # 4 f-tiles covering q|k (pair-major)
VW = NH * HD             # 256 v columns
NEG = -1.0e9

BF16 = mybir.dt.bfloat16
KDT = os.environ.get("KDT", "bf16")
MM_DT = BF16 if KDT == "bf16" else F32R  # matmul operand dtype
NP_MM = None  # set lazily (ml_dtypes import) in make_in_maps


def _mm(t):
    return t


def _bcast(ap, parts):
    """Partition-broadcast view of a [1, N] AP (stride-0 partition dim)."""
    return bass.AP(tensor=ap.tensor, offset=ap.offset,
                   ap=[[0, parts]] + list(ap.ap[1:]))


def _emit(tc, nc, xT_d, wT_d, owT_d, mask_d, out_d):
    import contextlib

    ctx = contextlib.ExitStack()
    with ctx:
        # ---------------- pools (all unscoped; PSUM = 2+3+3 = 8 banks) ----
        sb = ctx.enter_context(tc.tile_pool(name="sb", bufs=1))
        p_pool = ctx.enter_context(tc.tile_pool(name="psb", bufs=4))
        an_pool = ctx.enter_context(tc.tile_pool(name="attn_n", bufs=4))
        sm_pool = ctx.enter_context(tc.tile_pool(name="smalls", bufs=8))
        out_pool = ctx.enter_context(tc.tile_pool(name="outsb", bufs=3))
        ps_mm = ctx.enter_context(
            tc.tile_pool(name="ps_mm", bufs=2, space=MemorySpace.PSUM))
        ps_st = ctx.enter_context(
            tc.tile_pool(name="ps_st", bufs=3, space=MemorySpace.PSUM))
        ps_at = ctx.enter_context(
            tc.tile_pool(name="ps_at", bufs=3, space=MemorySpace.PSUM))

        # qT/kT: tile h in [0,4) = qT of head h in rows 0:64 (rows 64:128
        # zeroed so scores matmuls run K=128 full-row); tile 4+h = kT.
        qk_sb = [sb.tile([128, S], MM_DT, tag=f"qk{i}", name=f"qk{i}")
                 for i in range(2 * NH)]
        for t in qk_sb:
            nc.vector.memset(t[HD:128, :], 0.0)
        # v: per seq-tile [128, 4 heads, 65] (64 v cols + ones col)
        v_sb = [sb.tile([128, NH, HD + 1], MM_DT, tag=f"v{i}", name=f"v{i}")
                for i in range(N_ST)]
        mask_sb = sb.tile([128, 640], F32)
        ones_sb = sb.tile([128, NH], F32)
        nc.vector.memset(ones_sb, 1.0)
        owT_sb = [sb.tile([128, D], MM_DT, tag=f"ow{i}", name=f"ow{i}")
                  for i in range(NP)]
        xT_sb = [sb.tile([128, S], MM_DT, tag=f"x{i}", name=f"x{i}")
                 for i in range(N_DT)]
        wT_sb = [sb.tile([128, 3 * VW], MM_DT, tag=f"w{i}", name=f"w{i}")
                 for i in range(N_DT)]

        nc.sync.dma_start(out=mask_sb, in_=mask_d)
        for p in range(NP):
            nc.sync.dma_start(out=owT_sb[p], in_=owT_d[p * 128:(p + 1) * 128, :])
        for i in range(N_DT):
            nc.sync.dma_start(out=xT_sb[i], in_=xT_d[i * 128:(i + 1) * 128, :])
            nc.sync.dma_start(out=wT_sb[i], in_=wT_d[i * 128:(i + 1) * 128, :])

        # ---------------- phase 1: qkv projection ----------------
        # qT/kT: out[f 128, s 512] += wT[d, f].T @ xT[d, s]
        for f in range(FQK):
            for sch in range(N_CH):
                pss = ps_mm.tile([128, 512], F32, tag="mm", name="psqk")
                for d in range(N_DT):
                    nc.tensor.matmul(
                        pss,
                        wT_sb[d][:, f * 128:(f + 1) * 128],
                        xT_sb[d][:, sch * 512:(sch + 1) * 512],
                        start=(d == 0),
                        stop=(d == N_DT - 1),
                    )
                for hh in range(2):
                    nc.vector.tensor_copy(
                        qk_sb[2 * f + hh][0:HD, sch * 512:(sch + 1) * 512],
                        pss[hh * HD:(hh + 1) * HD, :])

        # v: out[s 128, 256] += xT[d, s].T @ wvT[d, 256]
        for st in range(N_ST):
            psvt = ps_mm.tile([128, VW], F32, tag="mm", name="psv")
            for d in range(N_DT):
                nc.tensor.matmul(
                    psvt,
                    xT_sb[d][:, st * 128:(st + 1) * 128],
                    wT_sb[d][:, 2 * VW:3 * VW],
                    start=(d == 0),
                    stop=(d == N_DT - 1),
                )
            nc.vector.tensor_copy(
                v_sb[st][:, :, 0:HD],
                psvt.rearrange("p (h d) -> p h d", h=NH),
            )
            nc.vector.tensor_copy(v_sb[st][:, :, HD:HD + 1], ones_sb)

        # ---------------- phase 2+3: attention + out projection ----------------
        # Emission is software-pipelined: chunk qc+1's attention is emitted
        # before chunk qc's out-projection so the in-order PE queue never
        # stalls on the (DVE+GpSimd) softmax-denominator normalize chain.
        an_hist = {}
        pend_norm = {}

        def emit_attention(qc):
            n_kt = 4 * (qc + 1)
            an = [an_pool.tile([128, 512], MM_DT, tag=f"an{p}", name=f"an{p}")
                  for p in range(NP)]
            an_hist[qc] = an
            for p in range(NP):
                at_ps = [ps_at.tile([HD + 1, 512], F32, tag="at", name="at_ps")
                         for _ in range(2)]
                # one-step software pipeline: scores/exp for kt overlap
                # the attnT accumulation of kt-1, so the in-order PE never
                # waits on the ACT exp chain.
                pend = {}
                for kt in range(n_kt + 1):
                    if kt < n_kt:
                        j = kt - 4 * qc  # >=0 on diagonal-crossing tiles
                        rs = 0 if j < 0 else min(j * 128, 256)
                        n = 512 - rs
                        for hp in range(2):
                            h = 2 * p + hp
                            st_ps = ps_st.tile([128, 512], F32, tag="st")
                            nc.tensor.matmul(
                                st_ps[:, 0:n],
                                qk_sb[NH + h][:, kt * 128:(kt + 1) * 128],
                                qk_sb[h][:, qc * 512 + rs:(qc + 1) * 512],
                                start=True,
                                stop=True,
                            )
                            if j >= 0:
                                # additive causal mask, nonzero region only
                                w = 128 if j < 3 else 256
                                mcol = 128 if j < 3 else 0
                                nc.vector.tensor_add(
                                    st_ps[:, 0:w], st_ps[:, 0:w],
                                    mask_sb[:, mcol:mcol + w],
                                )
                            p_sb = p_pool.tile([128, 512], MM_DT, tag="p")
                            nc.scalar.activation(p_sb[:, 0:n], st_ps[:, 0:n], EXP)
                            pend[(kt, hp)] = (p_sb, rs, n)
                    if kt >= 1:
                        for hp in range(2):
                            p_sb, rs, n = pend.pop((kt - 1, hp))
                            nc.tensor.matmul(
                                at_ps[hp][:, rs:512],
                                v_sb[kt - 1][:, 2 * p + hp, :],
                                p_sb[:, 0:n],
                                start=(kt == 1),
                                stop=(kt == n_kt),
                            )
                # Release the accumulator banks fast (two DVE copies);
                # the slow reciprocal chain is deferred to emit_normalize
                # so it never sits ahead of the next segment's DVE work.
                for hp in range(2):
                    anu = an_pool.tile([HD, 512], MM_DT, tag="anu")
                    nc.vector.tensor_copy(anu, at_ps[hp][0:HD, :])
                    lsb = sm_pool.tile([1, 512], F32, tag="lsb")
                    nc.vector.tensor_copy(lsb, at_ps[hp][HD:HD + 1, :])
                    pend_norm[(qc, p, hp)] = (anu, lsb)

        def emit_normalize(qc):
            an = an_hist[qc]
            for p in range(NP):
                for hp in range(2):
                    anu, lsb = pend_norm.pop((qc, p, hp))
                    rec = sm_pool.tile([1, 512], F32, tag="rec")
                    nc.vector.reciprocal_approx_fast(rec, lsb)
                    bc = sm_pool.tile([HD, 512], F32, tag="bc")
                    nc.gpsimd.partition_broadcast(bc, rec)
                    nc.vector.tensor_mul(
                        an[p][hp * HD:(hp + 1) * HD, :], anu, bc,
                    )

        def emit_outproj(qc):
            an = an_hist.pop(qc)
            for qs in range(4):
                qsl = slice(qs * 128, (qs + 1) * 128)
                for e in range(2):
                    ops = ps_mm.tile([128, 512], F32, tag="mm", name="psout")
                    for p in range(NP):
                        nc.tensor.matmul(
                            ops,
                            an[p][:, qsl],
                            owT_sb[p][:, e * 512:(e + 1) * 512],
                            start=(p == 0),
                            stop=(p == NP - 1),
                        )
                    osb = out_pool.tile([128, 512], F32, tag="osb", name="osb")
                    nc.vector.tensor_copy(osb, ops)
                    nc.sync.dma_start(
                        out=out_d[qc * 512 + qs * 128:qc * 512 + (qs + 1) * 128,
                                  e * 512:(e + 1) * 512],
                        in_=osb,
                    )

        # Chunk order: start with 1 (v tiles 0..7 ready early), end with 0
        # (shortest chunk) so the final normalize+out-proj tail is minimal.
        order = [1, 2, 3, 0]
        prev = None
        for qc in order:
            emit_attention(qc)
            if prev is not None:
                emit_normalize(prev)
                emit_outproj(prev)
            prev = qc
        emit_normalize(prev)
        emit_outproj(prev)


_CACHE = {}


def _build():
    if "nc" in _CACHE:
        return _CACHE["nc"]
    nc = bacc.Bacc("TRN2", target_bir_lowering=False, debug=False)
    xT_d = nc.dram_tensor("xT", [D, S], MM_DT, kind="ExternalInput").ap()
    wT_d = nc.dram_tensor("wT", [D, 3 * VW], MM_DT, kind="ExternalInput").ap()
    owT_d = nc.dram_tensor("owT", [VW, D], MM_DT, kind="ExternalInput").ap()
    mask_d = nc.dram_tensor("mask", [128, 640], F32, kind="ExternalInput").ap()
    out_d = nc.dram_tensor("out", [S, D], F32, kind="ExternalOutput").ap()
    with tile.TileContext(nc) as tc:
        _emit(tc, nc, xT_d, wT_d, owT_d, mask_d, out_d)
    nc.compile()
    _CACHE["nc"] = nc
    return nc


def _mask_np():
    # [128, 640]: cols 0-127 all NEG, cols 128-255 lower-triangular keep
    # (col >= row -> 0 else NEG), cols 256-639 zeros.
    m = np.zeros((128, 640), np.float32)
    m[:, 0:128] = NEG
    r = np.arange(128)
    tri = np.where(r[None, :] >= r[:, None], 0.0, NEG).astype(np.float32)
    m[:, 128:256] = tri
    return m


def make_in_maps(x, qkv_w, out_w):
    """Per-core input dicts for the 8-way (batch x head-group) sharding."""
    x = np.asarray(x, np.float32)
    qkv_w = np.asarray(qkv_w, np.float32)
    out_w = np.asarray(out_w, np.float32)
    xT = [np.ascontiguousarray(x[b].T) for b in range(B)]
    mask = _mask_np()
    import ml_dtypes
    np_mm = ml_dtypes.bfloat16 if KDT == "bf16" else np.float32
    in_maps = []
    for c in range(NCORES):
        b = c // 4
        h0 = (c % 4) * NH
        rows = np.arange(h0 * HD, (h0 + NH) * HD)
        wq = qkv_w[rows] * np.float32(SCALE)
        wk = qkv_w[D + rows]
        wv = qkv_w[2 * D + rows]
        wT = np.ascontiguousarray(np.concatenate([wq, wk, wv], 0).T)
        owT = np.ascontiguousarray(out_w[:, rows].T)
        in_maps.append({"xT": xT[b].astype(np_mm), "wT": wT.astype(np_mm), "owT": owT.astype(np_mm), "mask": mask})
    return in_maps


def kernel(x, qkv_w, out_w, _trace=False, _trace_cores=None):
    nc = _build()
    in_maps = make_in_maps(x, qkv_w, out_w)
    res = run_bass_kernel_spmd(
        nc, in_maps, core_ids=list(range(NCORES)),
        trace=_trace, trace_cores=_trace_cores,
    )
    outs = [r["out"] for r in res.results]
    full = np.stack([
        outs[0] + outs[1] + outs[2] + outs[3],
        outs[4] + outs[5] + outs[6] + outs[7],
    ]).astype(np.float32)
    if _trace:
        return full, res
    return full


# revision 22
# speedup vs baseline: 1.0297x; 1.0297x over previous
"""Causal multi-head attention block (qkv proj + causal softmax attention + out proj)
for Trainium2, sharded over 8 NeuronCores: data-parallel over batch (2) x
tensor-parallel over heads (4 heads per core of 16).

Each core computes, for its batch b and its 4 heads:
  qT,kT [hd, S] and v [S, hd]  (qkv projection, weights pre-transposed on host)
  ST    [k, q] = kT.T-scores transposed, causal-blocked
  P     = exp(ST + mask)
  attnT [hd+1, q] = [v | 1].T @ P   (row hd = softmax denominator)
  attn_n = attnT / denom
  out_partial [S, D] = attn_n.T @ owT  (row-parallel out proj)
Host sums the 4 per-core partials of each batch.
"""

import os
import sys

import numpy as np

sys.path.insert(0, "/opt/trn_rl_repo")

import concourse.bass as bass
import concourse.tile as tile
from concourse import bacc, mybir
from concourse.bass import MemorySpace
from concourse.bass_utils import run_bass_kernel_spmd

F32 = mybir.dt.float32
F32R = mybir.dt.float32r
EXP = mybir.ActivationFunctionType.Exp
LN = mybir.ActivationFunctionType.Ln

B, S, D = 2, 2048, 1024
H, HD = 16, 64
NCORES = 8
NH = 4          # heads per core
NP = 2          # head pairs per core
SCALE = HD ** -0.5

N_DT = D // 128          # 8 d-tiles of 128
N_ST = S // 128          # 16 seq tiles of 128
N_CH = S // 512          # 4 seq chunks of 512
FQK = 2 * NH * HD // 128  # 4 f-tiles covering q|k (pair-major)
VW = NH * HD             # 256 v columns
NEG = -1.0e9

BF16 = mybir.dt.bfloat16
KDT = os.environ.get("KDT", "bf16")
MM_DT = BF16 if KDT == "bf16" else F32R  # matmul operand dtype
NP_MM = None  # set lazily (ml_dtypes import) in make_in_maps


def _mm(t):
    return t


def _bcast(ap, parts):
    """Partition-broadcast view of a [1, N] AP (stride-0 partition dim)."""
    return bass.AP(tensor=ap.tensor, offset=ap.offset,
                   ap=[[0, parts]] + list(ap.ap[1:]))


def _emit(tc, nc, xT_d, wT_d, owT_d, mask_d, out_d):
    import contextlib

    ctx = contextlib.ExitStack()
    with ctx:
        # ---------------- pools (all unscoped; PSUM = 2+3+3 = 8 banks) ----
        sb = ctx.enter_context(tc.tile_pool(name="sb", bufs=1))
        p_pool = ctx.enter_context(tc.tile_pool(name="psb", bufs=4))
        an_pool = ctx.enter_context(tc.tile_pool(name="attn_n", bufs=4))
        sm_pool = ctx.enter_context(tc.tile_pool(name="smalls", bufs=8))
        out_pool = ctx.enter_context(tc.tile_pool(name="outsb", bufs=3))
        ps_mm = ctx.enter_context(
            tc.tile_pool(name="ps_mm", bufs=2, space=MemorySpace.PSUM))
        ps_st = ctx.enter_context(
            tc.tile_pool(name="ps_st", bufs=3, space=MemorySpace.PSUM))
        ps_at = ctx.enter_context(
            tc.tile_pool(name="ps_at", bufs=3, space=MemorySpace.PSUM))

        # qT/kT: tile h in [0,4) = qT of head h in rows 0:64 (rows 64:128
        # zeroed so scores matmuls run K=128 full-row); tile 4+h = kT.
        qk_sb = [sb.tile([128, S], MM_DT, tag=f"qk{i}", name=f"qk{i}")
                 for i in range(2 * NH)]
        for t in qk_sb:
            nc.vector.memset(t[HD:128, :], 0.0)
        # v: per seq-tile [128, 4 heads, 65] (64 v cols + ones col)
        v_sb = [sb.tile([128, NH, HD + 1], MM_DT, tag=f"v{i}", name=f"v{i}")
                for i in range(N_ST)]
        mask_sb = sb.tile([128, 640], F32)
        ones_sb = sb.tile([128, NH], F32)
        nc.vector.memset(ones_sb, 1.0)
        owT_sb = [sb.tile([128, D], MM_DT, tag=f"ow{i}", name=f"ow{i}")
                  for i in range(NP)]
        xT_sb = [sb.tile([128, S], MM_DT, tag=f"x{i}", name=f"x{i}")
                 for i in range(N_DT)]
        wT_sb = [sb.tile([128, 3 * VW], MM_DT, tag=f"w{i}", name=f"w{i}")
                 for i in range(N_DT)]

        nc.sync.dma_start(out=mask_sb, in_=mask_d)
        for p in range(NP):
            nc.sync.dma_start(out=owT_sb[p], in_=owT_d[p * 128:(p + 1) * 128, :])
        for i in range(N_DT):
            nc.sync.dma_start(out=xT_sb[i], in_=xT_d[i * 128:(i + 1) * 128, :])
            nc.sync.dma_start(out=wT_sb[i], in_=wT_d[i * 128:(i + 1) * 128, :])

        # ---------------- phase 1: qkv projection ----------------
        # qT/kT: out[f 128, s 512] += wT[d, f].T @ xT[d, s]
        for f in range(FQK):
            for sch in range(N_CH):
                pss = ps_mm.tile([128, 512], F32, tag="mm", name="psqk")
                for d in range(N_DT):
                    nc.tensor.matmul(
                        pss,
                        wT_sb[d][:, f * 128:(f + 1) * 128],
                        xT_sb[d][:, sch * 512:(sch + 1) * 512],
                        start=(d == 0),
                        stop=(d == N_DT - 1),
                    )
                for hh in range(2):
                    nc.vector.tensor_copy(
                        qk_sb[2 * f + hh][0:HD, sch * 512:(sch + 1) * 512],
                        pss[hh * HD:(hh + 1) * HD, :])

        # v: out[s 128, 256] += xT[d, s].T @ wvT[d, 256]
        for st in range(N_ST):
            psvt = ps_mm.tile([128, VW], F32, tag="mm", name="psv")
            for d in range(N_DT):
                nc.tensor.matmul(
                    psvt,
                    xT_sb[d][:, st * 128:(st + 1) * 128],
                    wT_sb[d][:, 2 * VW:3 * VW],
                    start=(d == 0),
                    stop=(d == N_DT - 1),
                )
            nc.vector.tensor_copy(
                v_sb[st][:, :, 0:HD],
                psvt.rearrange("p (h d) -> p h d", h=NH),
            )
            nc.vector.tensor_copy(v_sb[st][:, :, HD:HD + 1], ones_sb)

        # ---------------- phase 2+3: attention + out projection ----------------
        # Emission is software-pipelined: chunk qc+1's attention is emitted
        # before chunk qc's out-projection so the in-order PE queue never
        # stalls on the (DVE+GpSimd) softmax-denominator normalize chain.
        an_hist = {}
        pend_norm = {}

        def emit_attention(qc):
            n_kt = 4 * (qc + 1)
            an = [an_pool.tile([128, 512], MM_DT, tag=f"an{p}", name=f"an{p}")
                  for p in range(NP)]
            an_hist[qc] = an
            for p in range(NP):
                at_ps = [ps_at.tile([HD + 1, 512], F32, tag="at", name="at_ps")
                         for _ in range(2)]
                # one-step software pipeline: scores/exp for kt overlap
                # the attnT accumulation of kt-1, so the in-order PE never
                # waits on the ACT exp chain.
                pend = {}
                for kt in range(n_kt + 1):
                    if kt < n_kt:
                        j = kt - 4 * qc  # >=0 on diagonal-crossing tiles
                        rs = 0 if j < 0 else min(j * 128, 256)
                        n = 512 - rs
                        for hp in range(2):
                            h = 2 * p + hp
                            st_ps = ps_st.tile([128, 512], F32, tag="st")
                            nc.tensor.matmul(
                                st_ps[:, 0:n],
                                qk_sb[NH + h][:, kt * 128:(kt + 1) * 128],
                                qk_sb[h][:, qc * 512 + rs:(qc + 1) * 512],
                                start=True,
                                stop=True,
                            )
                            if j >= 0:
                                # additive causal mask, nonzero region only
                                w = 128 if j < 3 else 256
                                mcol = 128 if j < 3 else 0
                                nc.vector.tensor_add(
                                    st_ps[:, 0:w], st_ps[:, 0:w],
                                    mask_sb[:, mcol:mcol + w],
                                )
                            p_sb = p_pool.tile([128, 512], MM_DT, tag="p")
                            nc.scalar.activation(p_sb[:, 0:n], st_ps[:, 0:n], EXP)
                            pend[(kt, hp)] = (p_sb, rs, n)
                    if kt >= 1:
                        for hp in range(2):
                            p_sb, rs, n = pend.pop((kt - 1, hp))
                            nc.tensor.matmul(
                                at_ps[hp][:, rs:512],
                                v_sb[kt - 1][:, 2 * p + hp, :],
                                p_sb[:, 0:n],
                                start=(kt == 1),
                                stop=(kt == n_kt),
                            )
                # Release the accumulator banks fast (two DVE copies);
                # the slow reciprocal chain is deferred to emit_normalize
                # so it never sits ahead of the next segment's DVE work.
                for hp in range(2):
                    anu = an_pool.tile([HD, 512], MM_DT, tag="anu")
                    nc.vector.tensor_copy(anu, at_ps[hp][0:HD, :])
                    lsb = sm_pool.tile([1, 512], F32, tag="lsb")
                    nc.vector.tensor_copy(lsb, at_ps[hp][HD:HD + 1, :])
                    pend_norm[(qc, p, hp)] = (anu, lsb)

        def emit_normalize(qc):
            an = an_hist[qc]
            for p in range(NP):
                for hp in range(2):
                    anu, lsb = pend_norm.pop((qc, p, hp))
                    rec = sm_pool.tile([1, 512], F32, tag="rec")
                    nc.vector.reciprocal_approx_fast(rec, lsb)
                    bc = sm_pool.tile([HD, 512], F32, tag="bc")
                    nc.gpsimd.partition_broadcast(bc, rec)
                    nc.vector.tensor_mul(
                        an[p][hp * HD:(hp + 1) * HD, :], anu, bc,
                    )

        def emit_outproj(qc):
            an = an_hist.pop(qc)
            for qs in range(4):
                qsl = slice(qs * 128, (qs + 1) * 128)
                for e in range(2):
                    ops = ps_mm.tile([128, 512], F32, tag="mm", name="psout")
                    for p in range(NP):
                        nc.tensor.matmul(
                            ops,
                            an[p][:, qsl],
                            owT_sb[p][:, e * 512:(e + 1) * 512],
                            start=(p == 0),
                            stop=(p == NP - 1),
                        )
                    osb = out_pool.tile([128, 512], F32, tag="osb", name="osb")
                    nc.vector.tensor_copy(osb, ops)
                    nc.sync.dma_start(
                        out=out_d[qc * 512 + qs * 128:qc * 512 + (qs + 1) * 128,
                                  e * 512:(e + 1) * 512],
                        in_=osb,
                    )

        # Chunk order: start with 1 (v tiles 0..7 ready early), end with 0
        # (shortest chunk) so the final normalize+out-proj tail is minimal.
        order = [0, 2, 3, 1]
        prev = None
        for qc in order:
            emit_attention(qc)
            if prev is not None:
                emit_normalize(prev)
                emit_outproj(prev)
            prev = qc
        emit_normalize(prev)
        emit_outproj(prev)


_CACHE = {}


def _build():
    if "nc" in _CACHE:
        return _CACHE["nc"]
    nc = bacc.Bacc("TRN2", target_bir_lowering=False, debug=False)
    xT_d = nc.dram_tensor("xT", [D, S], MM_DT, kind="ExternalInput").ap()
    wT_d = nc.dram_tensor("wT", [D, 3 * VW], MM_DT, kind="ExternalInput").ap()
    owT_d = nc.dram_tensor("owT", [VW, D], MM_DT, kind="ExternalInput").ap()
    mask_d = nc.dram_tensor("mask", [128, 640], F32, kind="ExternalInput").ap()
    out_d = nc.dram_tensor("out", [S, D], F32, kind="ExternalOutput").ap()
    with tile.TileContext(nc) as tc:
        _emit(tc, nc, xT_d, wT_d, owT_d, mask_d, out_d)
    nc.compile()
    _CACHE["nc"] = nc
    return nc


def _mask_np():
    # [128, 640]: cols 0-127 all NEG, cols 128-255 lower-triangular keep
    # (col >= row -> 0 else NEG), cols 256-639 zeros.
    m = np.zeros((128, 640), np.float32)
    m[:, 0:128] = NEG
    r = np.arange(128)
    tri = np.where(r[None, :] >= r[:, None], 0.0, NEG).astype(np.float32)
    m[:, 128:256] = tri
    return m


def make_in_maps(x, qkv_w, out_w):
    """Per-core input dicts for the 8-way (batch x head-group) sharding."""
    x = np.asarray(x, np.float32)
    qkv_w = np.asarray(qkv_w, np.float32)
    out_w = np.asarray(out_w, np.float32)
    xT = [np.ascontiguousarray(x[b].T) for b in range(B)]
    mask = _mask_np()
    import ml_dtypes
    np_mm = ml_dtypes.bfloat16 if KDT == "bf16" else np.float32
    in_maps = []
    for c in range(NCORES):
        b = c // 4
        h0 = (c % 4) * NH
        rows = np.arange(h0 * HD, (h0 + NH) * HD)
        wq = qkv_w[rows] * np.float32(SCALE)
        wk = qkv_w[D + rows]
        wv = qkv_w[2 * D + rows]
        wT = np.ascontiguousarray(np.concatenate([wq, wk, wv], 0).T)
        owT = np.ascontiguousarray(out_w[:, rows].T)
        in_maps.append({"xT": xT[b].astype(np_mm), "wT": wT.astype(np_mm), "owT": owT.astype(np_mm), "mask": mask})
    return in_maps


def kernel(x, qkv_w, out_w, _trace=False, _trace_cores=None):
    nc = _build()
    in_maps = make_in_maps(x, qkv_w, out_w)
    res = run_bass_kernel_spmd(
        nc, in_maps, core_ids=list(range(NCORES)),
        trace=_trace, trace_cores=_trace_cores,
    )
    outs = [r["out"] for r in res.results]
    full = np.stack([
        outs[0] + outs[1] + outs[2] + outs[3],
        outs[4] + outs[5] + outs[6] + outs[7],
    ]).astype(np.float32)
    if _trace:
        return full, res
    return full
